# revision 2
# baseline (speedup 1.0000x reference)
"""AttentionDecoder (Bahdanau attention + GRU cell, 32 decode steps) on 8 TRN2 cores.

Data-parallel over batch: 4 batches per core, no collectives. The whole working
set (UaEnc projection, encoder outputs in two layouts) lives in SBUF, so the
32-step loop never touches HBM. Per step and per local batch b:

  S_b  = tanh(UaEnc_b + q_b)             ACT (128_h, 2048_t), bf16 out
  e_T  = chunk(S_b).T @ va               PE, 16 matmuls -> (128_t, 16_c) in PSUM
  w~   = exp(e_T)  [+ accum per lane]    ACT (no max-sub: |e| < ~45 is f32-safe)
  s    = ones128.T @ acc                 PE partition-reduce -> scalar
  R    = ones1x128.T @ (1/s)             PE broadcast to all partitions
  wn   = w~ * R                          DVE, bf16 out (normalized weights)
  ctx  = sum_c enc_TH_chunk(b,c) @ wn_c  PE accumulate -> (128_h, 1)
  GRU  : r/z via 0.5+0.5*tanh(x/2) (stays in exp_and_others ACT table set),
         n = tanh(A + tanh_r * B) with A/B host-folded weight combinations.

The 4 batch chains are independent; their softmax/GRU tails hide under the
other batches' ACT tanh work (ACT is the bottleneck engine).
"""

import sys
import numpy as np

for _p in ("/opt/trn_rl_repo",):
    if _p not in sys.path:
        sys.path.insert(0, _p)

B, T, H = 32, 2048, 128
OUT_LEN = 32
NCORES = 8
BL = B // NCORES          # 4 local batches per core
NCH = T // H              # 16 chunks of 128 along T

_cache = {}


def _build():
    from concourse import bass, bacc, tile

    mybir = bass.mybir
    F32 = mybir.dt.float32
    BF16 = mybir.dt.bfloat16
    Act = mybir.ActivationFunctionType
    Alu = mybir.AluOpType

    nc = bacc.Bacc(None, target_bir_lowering=False, debug=False)

    enc_d = nc.declare_dram_parameter("enc", [BL, T, H], F32, isOutput=False)
    uat_d = nc.declare_dram_parameter("UaT", [H, H], F32, isOutput=False)
    wat_d = nc.declare_dram_parameter("WaT", [H, H], F32, isOutput=False)
    gw_d = nc.declare_dram_parameter("GW", [H, 7 * H], F32, isOutput=False)
    bias_d = nc.declare_dram_parameter("BIAS", [1, 4 * H], F32, isOutput=False)
    va_d = nc.declare_dram_parameter("va", [H, 1], F32, isOutput=False)
    id_d = nc.declare_dram_parameter("ID", [H, H], F32, isOutput=False)
    outh_d = nc.declare_dram_parameter("out_h", [OUT_LEN, BL, H], F32, isOutput=True)
    outw_d = nc.declare_dram_parameter("out_w", [BL, T], F32, isOutput=True)

    with tile.TileContext(nc) as tc:
        with (
            tc.tile_pool(name="pers", bufs=1) as pers,
            tc.tile_pool(name="work", bufs=4) as work,
            tc.tile_pool(name="hpool", bufs=8) as hpool,
            tc.tile_pool(name="psum", bufs=2, space="PSUM") as psum,
        ):
            # ---------- persistent tiles ----------
            enc_th_f = pers.tile([H, BL * T], F32, tag="enc_th_f")
            enc_th_b = pers.tile([H, BL * T], BF16, tag="enc_th_b")
            enc_ht = pers.tile([H, BL * T], F32, tag="enc_ht")
            uaenc = pers.tile([H, BL * T], F32, tag="uaenc")
            uat_sb = pers.tile([H, H], F32, tag="uat")
            wat_sb = pers.tile([H, H], F32, tag="wat")
            id_sb = pers.tile([H, H], F32, tag="idm")
            gw_f = pers.tile([H, 7 * H], F32, tag="gw_f")
            gw_b = pers.tile([H, 7 * H], BF16, tag="gw_b")
            bias_sb = pers.tile([1, 4 * H], F32, tag="bias")
            va_f = pers.tile([H, 1], F32, tag="va_f")
            va_b = pers.tile([H, 1], BF16, tag="va_b")
            ones_k = pers.tile([H, 1], F32, tag="ones_k")
            ones_m = pers.tile([1, H], F32, tag="ones_m")
            ones_1 = pers.tile([1, 1], F32, tag="ones_1")
            zero_f = pers.tile([H, BL], F32, tag="zero_f")
            zero_b16 = pers.tile([H, BL], BF16, tag="zero_b16")
            out_all = pers.tile([H, OUT_LEN * BL], F32, tag="out_all")
            wlast = pers.tile([H, BL * NCH], F32, tag="wlast")

            # ---------- load weights ----------
            nc.sync.dma_start(out=uat_sb[:], in_=uat_d[:])
            nc.sync.dma_start(out=wat_sb[:], in_=wat_d[:])
            nc.sync.dma_start(out=id_sb[:], in_=id_d[:])
            nc.sync.dma_start(out=gw_f[:], in_=gw_d[:])
            nc.sync.dma_start(out=bias_sb[:], in_=bias_d[:])
            nc.sync.dma_start(out=va_f[:], in_=va_d[:])
            nc.vector.tensor_copy(gw_b[:], gw_f[:])
            nc.vector.tensor_copy(va_b[:], va_f[:])
            nc.vector.memset(ones_k[:], 1.0)
            nc.vector.memset(ones_m[:], 1.0)
            nc.vector.memset(ones_1[:], 1.0)
            nc.vector.memset(zero_f[:], 0.0)
            nc.vector.memset(zero_b16[:], 0.0)

            # ---------- prologue: load enc, transpose, project ----------
            for b in range(BL):
                dst = enc_th_f[:, b * T:(b + 1) * T].rearrange(
                    "p (c q) -> p c q", c=NCH)
                src = enc_d[b].rearrange("(c p) h -> p c h", p=H)
                nc.sync.dma_start(out=dst, in_=src)
                nc.vector.tensor_copy(
                    enc_th_b[:, b * T:(b + 1) * T], enc_th_f[:, b * T:(b + 1) * T])
                for c in range(NCH):
                    ptr = psum.tile([H, H], F32, tag="pa")
                    nc.tensor.transpose(
                        ptr[:],
                        enc_th_f[:, (b * NCH + c) * H:(b * NCH + c + 1) * H],
                        id_sb[:])
                    col = b * T + c * H
                    if c % 2 == 0:
                        nc.scalar.copy(enc_ht[:, col:col + H], ptr[:])
                    else:
                        nc.vector.tensor_copy(enc_ht[:, col:col + H], ptr[:])
                for cc in range(4):
                    pue = psum.tile([H, 512], F32, tag="pb")
                    nc.tensor.matmul(
                        pue[:], uat_sb[:],
                        enc_ht[:, b * T + cc * 512: b * T + (cc + 1) * 512],
                        start=True, stop=True)
                    col = b * T + cc * 512
                    if cc % 2 == 0:
                        nc.scalar.copy(uaenc[:, col:col + 512], pue[:])
                    else:
                        nc.vector.tensor_copy(uaenc[:, col:col + 512], pue[:])

            # ---------- decode loop ----------
            h_f = [zero_f[:, b:b + 1] for b in range(BL)]      # f32 APs
            h_b16 = [zero_b16[:, b:b + 1] for b in range(BL)]  # bf16 APs
            q_hold = [None] * BL

            for s in range(OUT_LEN):
                for b in range(BL):
                    # --- attention scores ---
                    S = work.tile([H, T], BF16, tag="S")
                    if q_hold[b] is None:
                        nc.scalar.activation(
                            S[:], uaenc[:, b * T:(b + 1) * T], Act.Tanh)
                    else:
                        nc.scalar.activation(
                            S[:], uaenc[:, b * T:(b + 1) * T], Act.Tanh,
                            bias=q_hold[b][:])
                    pe_t = psum.tile([H, NCH], F32, tag="pa")
                    for c in range(NCH):
                        nc.tensor.matmul(
                            pe_t[:, c:c + 1], S[:, c * H:(c + 1) * H], va_b[:],
                            start=True, stop=True)
                    # --- softmax ---
                    wt = work.tile([H, NCH], F32, tag="wt")
                    acc = work.tile([H, 1], F32, tag="acc")
                    nc.scalar.activation(wt[:], pe_t[:], Act.Exp,
                                         accum_out=acc[:])
                    p_sr = psum.tile([H, 2], F32, tag="pc")
                    nc.tensor.matmul(p_sr[0:1, 0:1], ones_k[:], acc[:],
                                     start=True, stop=True)
                    rsum = work.tile([1, 1], F32, tag="rsum")
                    nc.vector.reciprocal(rsum[:], p_sr[0:1, 0:1])
                    nc.tensor.matmul(p_sr[:, 1:2], ones_m[:], rsum[:],
                                     start=True, stop=True)
                    wn = work.tile([H, NCH], BF16, tag="wn")
                    nc.vector.tensor_mul(
                        wn[:], wt[:], p_sr[:, 1:2].broadcast_to((H, NCH)))
                    if s == OUT_LEN - 1:
                        nc.vector.tensor_mul(
                            wlast[:, b * NCH:(b + 1) * NCH], wt[:],
                            p_sr[:, 1:2].broadcast_to((H, NCH)))
                    # --- context (bf16 matmuls, accumulated) ---
                    p_ctx = psum.tile([H, 1], F32, tag="pb")
                    for c in range(NCH):
                        nc.tensor.matmul(
                            p_ctx[:],
                            enc_th_b[:, (b * NCH + c) * H:(b * NCH + c + 1) * H],
                            wn[:, c:c + 1],
                            start=(c == 0), stop=(c == NCH - 1))
                    ctx16 = work.tile([H, 1], BF16, tag="ctx16")
                    nc.vector.tensor_copy(ctx16[:], p_ctx[:])
                    # --- GRU gate pre-activations: cols r=0, z=1, A=2, B=3 ---
                    pg = psum.tile([H, 5], F32, tag="pd")
                    for g, (wh, wc, bi) in enumerate(
                            ((0, 1, 0), (2, 3, 1), (4, 5, 2), (6, None, 3))):
                        nc.tensor.matmul(pg[:, g:g + 1],
                                         gw_b[:, wh * H:(wh + 1) * H],
                                         h_b16[b], start=True, stop=False)
                        if wc is not None:
                            nc.tensor.matmul(pg[:, g:g + 1],
                                             gw_b[:, wc * H:(wc + 1) * H],
                                             ctx16[:], start=False, stop=False)
                        nc.tensor.matmul(pg[:, g:g + 1],
                                         bias_sb[:, bi * H:(bi + 1) * H],
                                         ones_1[:], start=False, stop=True)
                    # r/z: sigmoid(x) = 0.5 + 0.5*tanh(x/2)
                    trz = work.tile([H, 2], F32, tag="trz")
                    nc.scalar.activation(trz[:], pg[:, 0:2], Act.Tanh, scale=0.5)
                    z_f = work.tile([H, 1], F32, tag="z_f")
                    nc.vector.tensor_scalar(z_f[:], trz[:, 1:2], 0.5, 0.5,
                                            Alu.mult, Alu.add)
                    v1 = work.tile([H, 1], F32, tag="v1")
                    nc.vector.tensor_mul(v1[:], trz[:, 0:1], pg[:, 3:4])
                    npre = work.tile([H, 1], F32, tag="npre")
                    nc.vector.tensor_add(npre[:], v1[:], pg[:, 2:3])
                    n_t = work.tile([H, 1], F32, tag="n_t")
                    nc.scalar.activation(n_t[:], npre[:], Act.Tanh)
                    # h_new = n + z*(h - n)
                    u_t = work.tile([H, 1], F32, tag="u_t")
                    nc.vector.tensor_sub(u_t[:], h_f[b], n_t[:])
                    v_t = work.tile([H, 1], F32, tag="v_t")
                    nc.vector.tensor_mul(v_t[:], z_f[:], u_t[:])
                    h_new = hpool.tile([H, 1], F32, tag="h")
                    nc.vector.tensor_add(h_new[:], n_t[:], v_t[:])
                    col = s * BL + b
                    nc.scalar.copy(out_all[:, col:col + 1], h_new[:])
                    hb16 = hpool.tile([H, 1], BF16, tag="hb")
                    nc.vector.tensor_copy(hb16[:], h_new[:])
                    h_f[b] = h_new[:]
                    h_b16[b] = hb16[:]
                    # q for next step
                    if s < OUT_LEN - 1:
                        pq = psum.tile([H, 1], F32, tag="pc")
                        nc.tensor.matmul(pq[:], wat_sb[:], h_new[:],
                                         start=True, stop=True)
                        qn = hpool.tile([H, 1], F32, tag="q")
                        nc.scalar.copy(qn[:], pq[:])
                        q_hold[b] = qn

            # ---------- outputs ----------
            p_oh = psum.tile([H, H], F32, tag="pa")
            nc.tensor.transpose(p_oh[:], out_all[:], id_sb[:])
            oh_sb = pers.tile([H, H], F32, tag="oh")
            nc.vector.tensor_copy(oh_sb[:], p_oh[:])
            nc.sync.dma_start(out=outh_d[:], in_=oh_sb[:])
            for b in range(BL):
                p_w = psum.tile([NCH, H], F32, tag="pb")
                nc.tensor.transpose(p_w[:], wlast[:, b * NCH:(b + 1) * NCH],
                                    id_sb[:])
                w_sb = work.tile([NCH, H], F32, tag="w_sb")
                nc.vector.tensor_copy(w_sb[:], p_w[:])
                nc.sync.dma_start(out=outw_d[b:b + 1, :], in_=w_sb[:])

    nc.compile()
    return nc


def _prep_weights(Wa, Ua, va, W_ih, W_hh, b_ih, b_hh):
    UaT = np.ascontiguousarray(Ua.T)
    WaT = np.ascontiguousarray(Wa.T)
    Wr_i, Wz_i, Wn_i = W_ih[0:H], W_ih[H:2 * H], W_ih[2 * H:3 * H]
    Wr_h, Wz_h, Wn_h = W_hh[0:H], W_hh[H:2 * H], W_hh[2 * H:3 * H]
    blocks = [
        (Wr_i[:, 0:H] + Wr_h).T,
        Wr_i[:, H:2 * H].T,
        (Wz_i[:, 0:H] + Wz_h).T,
        Wz_i[:, H:2 * H].T,
        (Wn_i[:, 0:H] + 0.5 * Wn_h).T,
        Wn_i[:, H:2 * H].T,
        (0.5 * Wn_h).T,
    ]
    GW = np.ascontiguousarray(np.concatenate(blocks, axis=1), dtype=np.float32)
    b_r = b_ih[0:H] + b_hh[0:H]
    b_z = b_ih[H:2 * H] + b_hh[H:2 * H]
    b_A = b_ih[2 * H:3 * H] + 0.5 * b_hh[2 * H:3 * H]
    b_B = 0.5 * b_hh[2 * H:3 * H]
    BIAS = np.concatenate([b_r, b_z, b_A, b_B])[None, :].astype(np.float32)
    ID = np.eye(H, dtype=np.float32)
    return UaT, WaT, GW, BIAS, ID


def kernel(**inputs):
    from concourse.bass_utils import run_bass_kernel_spmd

    enc = np.asarray(inputs["encoder_outputs"], dtype=np.float32)
    Wa = np.asarray(inputs["Wa"], dtype=np.float32)
    Ua = np.asarray(inputs["Ua"], dtype=np.float32)
    va = np.asarray(inputs["va"], dtype=np.float32)
    W_ih = np.asarray(inputs["W_ih"], dtype=np.float32)
    W_hh = np.asarray(inputs["W_hh"], dtype=np.float32)
    b_ih = np.asarray(inputs["b_ih"], dtype=np.float32)
    b_hh = np.asarray(inputs["b_hh"], dtype=np.float32)
    out_len = int(np.asarray(inputs["out_len"]))
    assert out_len == OUT_LEN and enc.shape == (B, T, H)

    UaT, WaT, GW, BIAS, ID = _prep_weights(Wa, Ua, va, W_ih, W_hh, b_ih, b_hh)

    if "nc" not in _cache:
        _cache["nc"] = _build()
    nc = _cache["nc"]

    in_maps = []
    for c in range(NCORES):
        in_maps.append({
            "enc": np.ascontiguousarray(enc[c * BL:(c + 1) * BL]),
            "UaT": UaT, "WaT": WaT, "GW": GW, "BIAS": BIAS,
            "va": np.ascontiguousarray(va[:, None]), "ID": ID,
        })
    res = run_bass_kernel_spmd(nc, in_maps, core_ids=list(range(NCORES)),
                               **_cache.get("run_kwargs", {}))
    _cache["last_results"] = res
    outs = np.stack([r["out_h"] for r in res.results], axis=1)  # (32,8,4,128)
    output = np.ascontiguousarray(outs.reshape(OUT_LEN * B, 1, H))
    w = np.concatenate([r["out_w"] for r in res.results], axis=0)
    return output, w
